# revision 17
# baseline (speedup 1.0000x reference)
"""MultiHeadCrossAttention kernel for 8 trn2 NeuronCores.

Reference computation (fp32, per batch b):
    q = Q[b] @ W_q.T ; k = K[b] @ W_k.T ; v = V[b] @ W_v.T      (heads on columns)
    per head h: S = (q_h @ k_h.T) / 8 ; E = exp(S); A = E / E.sum(-1)
    out[b] = concat_h(A @ v_h) @ W_o.T ; rows with mask==0 zeroed

Sharding: 8 cores = (batch b in {0,1}) x (head-group hg in {0..3}, 4 heads each).
Each core computes a partial output  out_part[b] = concat(heads hg) @ W_o[:, cols].T
and the host sums the 4 partials per batch.

Schedule (v4): jointly limited by the PE (~137us of matmul stream) and the
Scalar engine (~137us of exp).  Dense same-shape matmul phases run at
~1.0x N-cycles while finely interleaved ones run ~20-100% slower, so the
big projection phases stay dense; only the lag-tolerant pieces (v-proj
chunks, later q-blocks) are emitted as units inside the attention loop:
  - DMAs split across both HWDGE rings in consumption order (K path first).
  - Pre-attention: full k-projection (dense), first q-block per head pair,
    v chunks 0-1.  exp starts ~30us in (vs ~60 for proj-everything-first).
  - v chunks 2-15 one per iteration (PV tolerates lag via the e-tile pool);
    remaining six q-blocks at iterations chosen well before their consumers.
  - W_o groups drain one per 4 iterations to avoid PE bursts at block
    boundaries.
  - Projection PSUM->SBUF copies run on Vector, never Scalar (exp only).

Per-core kernel layout (all matmul operands bf16, fp32 PSUM accumulation):
  - xqT/xkT/xvT: host-transposed [1024(in), 2048(seq)] so the contraction dim
    (model dim) lands on SBUF partitions.
  - qT/kT stored as [128, 2, 2048]: partition = local-head-dim % 128. Head pair
    hp occupies chunk hp; even head rows 0:64, odd head rows 64:128 - so the two
    scores matmuls of a pair use disjoint PE row groups and run concurrently.
  - Scores are computed transposed, ST[kpos, q], per 128-kpos chunk j; exp runs
    on ScalarE over a 2-bank [128, 1024] PSUM region (even|odd head halves) with
    the 1/8 scale fused.
  - PV: acc[128, 512] += [ones | 0*63 | v_h].T @ E  accumulated over j. Row 0
    is the softmax denominator; rows 64:128 the unnormalized output ([d, q]).
  - Normalize: copy acc to SBUF (frees the PSUM bank fast), approx reciprocal
    of row 0, broadcast across partitions via a DRAM bounce + step-0 read (or a
    PE outer-product for the final pair, keeping the last W_o off the DMA
    latency), one tensor_mul into the W_o lhsT layout (bf16).
  - W_o: final[q,:] accumulated over the 256 local dims; the mask (per-q 0/1)
    is applied by the PSUM->SBUF tensor_scalar multiply before the output DMA.
"""

import numpy as np
import ml_dtypes

import concourse.bass as bass
import concourse.bacc as bacc
import concourse.mybir as mybir
import concourse.tile as tile
from concourse.tile import add_dep_helper
from contextlib import ExitStack

F32 = mybir.dt.float32
BF16 = mybir.dt.bfloat16
AF = mybir.ActivationFunctionType

B = 2
SEQ = 2048          # Sq == Sk
D = 1024            # model dim
DL = 256            # local head dims per core (4 heads x 64)
HL = 4              # local heads
DH = 64             # head dim
NCORES = 8

_PROGRAM = None


def build_program():
    nc = bacc.Bacc("TRN2", target_bir_lowering=False)

    xqT = nc.declare_dram_parameter("xqT", [D, SEQ], BF16, isOutput=False)
    xkT = nc.declare_dram_parameter("xkT", [D, SEQ], BF16, isOutput=False)
    xvT = nc.declare_dram_parameter("xvT", [D, SEQ], BF16, isOutput=False)
    wq = nc.declare_dram_parameter("wq", [D, DL], BF16, isOutput=False)
    wk = nc.declare_dram_parameter("wk", [D, DL], BF16, isOutput=False)
    wv = nc.declare_dram_parameter("wv", [D, DL], BF16, isOutput=False)
    wo = nc.declare_dram_parameter("wo", [DL, D], BF16, isOutput=False)
    maskf = nc.declare_dram_parameter("maskf", [128, SEQ // 128], F32, isOutput=False)
    out_part = nc.declare_dram_parameter("out_part", [SEQ, D], F32, isOutput=True)

    r_dram = nc.dram_tensor("r_bounce", [16, 512], F32)  # recip bounce rows

    with tile.TileContext(nc) as tc, ExitStack() as ctx:
        const = ctx.enter_context(tc.tile_pool(name="const", bufs=1))
        proj = ctx.enter_context(tc.tile_pool(name="proj", bufs=1))
        xpool = ctx.enter_context(tc.tile_pool(name="xpool", bufs=10))
        epool = ctx.enter_context(tc.tile_pool(name="epool", bufs=8))
        opool = ctx.enter_context(tc.tile_pool(name="opool", bufs=4))
        ospool = ctx.enter_context(tc.tile_pool(name="ospool", bufs=4))
        rpool = ctx.enter_context(tc.tile_pool(name="rpool", bufs=4))
        pp = ctx.enter_context(tc.tile_pool(name="pp", bufs=2, space="PSUM"))
        stp = ctx.enter_context(tc.tile_pool(name="stp", bufs=2, space="PSUM"))
        accp = ctx.enter_context(tc.tile_pool(name="accp", bufs=2, space="PSUM"))

        # ---------------- constants + inputs, consumption-ordered ----------
        wk_sb = const.tile([128, 8, DL], BF16)
        wq_sb = const.tile([128, 8, DL], BF16)
        wv_sb = const.tile([128, 8, DL], BF16)
        wo_sb = const.tile([128, 2, D], BF16)
        mask_sb = const.tile([128, SEQ // 128], F32)
        ones_sb = const.tile([1, 64], F32)

        def dma_x(xT, half, quarter, name, eng):
            x_t = xpool.tile([128, 4, 1024], BF16, tag="x", name=name)
            eng.dma_start(
                x_t[:],
                xT[
                    quarter * 512 : (quarter + 1) * 512,
                    half * 1024 : (half + 1) * 1024,
                ].rearrange("(a p) q -> p a q", p=128),
            )
            return x_t

        # quarter 0 on the sync ring, quarter 1 on the scalar ring.
        nc.scalar.dma_start(wk_sb[:], wk[:].rearrange("(a p) d -> p a d", p=128))
        xk = [
            [dma_x(xkT, 0, 0, "xk00", nc.sync), dma_x(xkT, 0, 1, "xk01", nc.scalar)],
            [dma_x(xkT, 1, 0, "xk10", nc.sync), dma_x(xkT, 1, 1, "xk11", nc.scalar)],
        ]
        nc.scalar.dma_start(wq_sb[:], wq[:].rearrange("(a p) d -> p a d", p=128))
        xq0 = [dma_x(xqT, 0, 0, "xq00", nc.sync), dma_x(xqT, 0, 1, "xq01", nc.scalar)]
        nc.scalar.dma_start(wv_sb[:], wv[:].rearrange("(a p) d -> p a d", p=128))
        xv = [
            [dma_x(xvT, 0, 0, "xv00", nc.sync), dma_x(xvT, 0, 1, "xv01", nc.scalar)],
            [dma_x(xvT, 1, 0, "xv10", nc.sync), dma_x(xvT, 1, 1, "xv11", nc.scalar)],
        ]
        nc.scalar.dma_start(wo_sb[:], wo[:].rearrange("(a p) d -> p a d", p=128))
        nc.scalar.dma_start(mask_sb[:], maskf[:])
        xq1 = [dma_x(xqT, 1, 0, "xq10", nc.sync), dma_x(xqT, 1, 1, "xq11", nc.scalar)]
        nc.vector.memset(ones_sb[:], 1.0)

        kT0_sb = proj.tile([128, SEQ], BF16)
        kT1_sb = proj.tile([128, SEQ], BF16)
        kTs = (kT0_sb, kT1_sb)
        qTs = [
            [proj.tile([128, 1024], BF16, name=f"qT{dm}_{h}") for h in range(2)]
            for dm in range(2)
        ]
        xqs = (xq0, xq1)
        vaugs = [
            proj.tile([128, HL, 128], BF16, name=f"vaug{j}") for j in range(16)
        ]
        for j in range(16):
            nc.vector.memset(vaugs[j][:], 0.0)
            nc.vector.memset(vaugs[j][:, :, 0:1], 1.0)

        # ---------------- projection emitters ----------------
        # Units emitted inside the attention loop are PACED behind the
        # current exp instruction (scheduler-only dep, no runtime sem), so
        # the list scheduler cannot commit a burst of unit matmuls to the
        # in-order PE stream ahead of upcoming scores matmuls.
        pace_after = [None]

        def pace(mm):
            if pace_after[0] is not None:
                add_dep_helper(
                    mm.ins, pace_after[0].ins, sync=False,
                    reason="pace projection unit behind exp stream",
                )

        # dense 512-col k/q block: dst[:, dst_col:dst_col+512] over 8 ki.
        def emit_kq_block(w_sb, x_pair, qc, dm, dst, dst_col, nm):
            ps = pp.tile([128, 512], F32, tag="pp", name=f"kq{nm}")
            for ki in range(8):
                mm = nc.tensor.matmul(
                    ps[:],
                    lhsT=w_sb[:, ki, dm * 128 : (dm + 1) * 128],
                    rhs=x_pair[ki // 4][:, ki % 4, qc * 512 : (qc + 1) * 512],
                    start=(ki == 0),
                    stop=(ki == 7),
                )
                if ki == 0:
                    pace(mm)
            nc.vector.tensor_copy(dst[:, dst_col : dst_col + 512], ps[:])

        # v chunk unit: vaug[base+km] for all 4 local heads.
        def emit_v_unit(half, km):
            x_pair = xv[half]
            ps = pp.tile([128, 512], F32, tag="pp", name=f"v{half}_{km}")
            for ki in range(8):
                mm = nc.tensor.matmul(
                    ps[:, 0:DL],
                    lhsT=x_pair[ki // 4][:, ki % 4, km * 128 : (km + 1) * 128],
                    rhs=wv_sb[:, ki, :],
                    start=(ki == 0),
                    stop=(ki == 7),
                )
                if ki == 0:
                    pace(mm)
            nc.vector.tensor_copy(
                vaugs[half * 8 + km][:, :, 64 : 64 + DH],
                ps[:, 0:DL].rearrange("p (h d) -> p h d", h=HL),
            )

        # ---------------- pre-attention (dense) ----------------
        # k projection for head pair 0 only (kT1 is not consumed until g64)
        for half in range(2):
            for qc in range(2):
                emit_kq_block(
                    wk_sb, xk[half], qc, 0, kTs[0],
                    half * 1024 + qc * 512, f"k{half}0{qc}",
                )
        # first q block for head pair 0 (q 0:512)
        emit_kq_block(wq_sb, xq0, 0, 0, qTs[0][0], 0, "q000")
        # v chunks 0, 1
        emit_v_unit(0, 0)
        emit_v_unit(0, 1)

        # ---------------- scheduled units ----------------
        # g = hp*64 + qp*16 + j; unit emitted at END of iteration g, always
        # before its first consumer's emission.
        sched = {}

        def at(g, fn):
            sched.setdefault(g, []).append(fn)

        for km in range(2, 8):      # vaug 2..7, consumed by PV hp0 at g=km
            at(km - 2, lambda km=km: emit_v_unit(0, km))
        for km in range(8):         # vaug 8..15, consumed at g=8+km
            at(6 + km, lambda km=km: emit_v_unit(1, km))
        # qT0 rest: consumed at qp1/qp2/qp3 of phase 1 (g16/g32/g48)
        at(4, lambda: emit_kq_block(wq_sb, xq0, 1, 0, qTs[0][0], 512, "q010"))
        at(20, lambda: emit_kq_block(wq_sb, xq1, 0, 0, qTs[0][1], 0, "q100"))
        at(36, lambda: emit_kq_block(wq_sb, xq1, 1, 0, qTs[0][1], 512, "q110"))
        # kT1: consumed from g64 (phase 2), kpos quarter at g64/g68/g72/g76
        at(24, lambda: emit_kq_block(wk_sb, xk[0], 0, 1, kTs[1], 0, "k100"))
        at(28, lambda: emit_kq_block(wk_sb, xk[0], 1, 1, kTs[1], 512, "k101"))
        at(40, lambda: emit_kq_block(wk_sb, xk[1], 0, 1, kTs[1], 1024, "k110"))
        at(44, lambda: emit_kq_block(wk_sb, xk[1], 1, 1, kTs[1], 1536, "k111"))
        # qT1: consumed at g64/g80/g96/g112
        at(48, lambda: emit_kq_block(wq_sb, xq0, 0, 1, qTs[1][0], 0, "q001"))
        at(52, lambda: emit_kq_block(wq_sb, xq0, 1, 1, qTs[1][0], 512, "q011"))
        at(56, lambda: emit_kq_block(wq_sb, xq1, 0, 1, qTs[1][1], 0, "q101"))
        at(60, lambda: emit_kq_block(wq_sb, xq1, 1, 1, qTs[1][1], 512, "q111"))

        # ---------------- attention + output projection ----------------
        def emit_wo_group(qp, outT_sb, mq):
            qg = qp * 4 + mq
            o_sb = ospool.tile([128, 1024], F32, tag="o", name=f"wo_o{qp}_{mq}")
            ps = [
                pp.tile([128, 512], F32, tag="pp", name=f"wops{qp}_{mq}_{oc}")
                for oc in range(2)
            ]
            for kc in range(2):
                for oc in range(2):
                    nc.tensor.matmul(
                        ps[oc][:],
                        lhsT=outT_sb[:, kc, mq * 128 : (mq + 1) * 128],
                        rhs=wo_sb[:, kc, oc * 512 : (oc + 1) * 512],
                        start=(kc == 0),
                        stop=(kc == 1),
                    )
            for oc in range(2):
                nc.vector.tensor_scalar_mul(
                    o_sb[:, oc * 512 : (oc + 1) * 512],
                    ps[oc][:],
                    mask_sb[:, qg : qg + 1],
                )
            nc.sync.dma_start(out_part[qg * 128 : (qg + 1) * 128, :], o_sb[:])

        pending_wo = []
        outTs = {}

        for hp in range(2):  # head pairs (2*hp, 2*hp+1) -- OUTER
            for qp in range(4):  # 512-query blocks
                q0 = qp * 512
                if hp == 0:
                    outTs[qp] = opool.tile(
                        [128, 2, 512], BF16, tag="outT", name=f"outT{qp}"
                    )
                outT_sb = outTs[qp]
                acc = [
                    accp.tile([128, 512], F32, tag="acc", name=f"acc{qp}_{hp}_{i}")
                    for i in range(2)
                ]
                for j in range(16):  # 128-key chunks
                    g = hp * 64 + qp * 16 + j
                    st = stp.tile([128, 1024], F32, tag="st")
                    for hi in range(2):  # even/odd head -> PE rows 0:64 / 64:128
                        r0 = hi * 64
                        nc.tensor.matmul(
                            st[:, hi * 512 : (hi + 1) * 512],
                            lhsT=kTs[hp][r0 : r0 + 64, j * 128 : (j + 1) * 128],
                            rhs=qTs[hp][q0 // 1024][
                                r0 : r0 + 64, (q0 % 1024) : (q0 % 1024) + 512
                            ],
                            start=True,
                            stop=True,
                        )
                    e_t = epool.tile([128, 1024], BF16, tag="e")
                    act = nc.scalar.activation(
                        out=e_t[:], in_=st[:], func=AF.Exp, scale=0.125
                    )
                    pace_after[0] = act
                    for hi in range(2):
                        h = 2 * hp + hi
                        nc.tensor.matmul(
                            acc[hi][:],
                            lhsT=vaugs[j][:, h, :],
                            rhs=e_t[:, hi * 512 : (hi + 1) * 512],
                            start=(j == 0),
                            stop=(j == 15),
                        )
                    for fn in sched.pop(g, ()):
                        fn()
                    if j % 4 == 1 and pending_wo:
                        emit_wo_group(*pending_wo.pop(0))
                if qp == 3 and hp == 1:
                    # tail: split the last normalize + W_o into two 256-q
                    # halves so W_o starts as soon as its half is ready; the
                    # reciprocal broadcast runs on-chip (PE outer product) to
                    # skip the DRAM round trip.
                    acc_sbs = []
                    for hi in range(2):
                        acc_sb = rpool.tile([128, 512], F32, tag="accsb")
                        nc.vector.tensor_copy(acc_sb[:], acc[hi][:])
                        acc_sbs.append(acc_sb)
                    for half in range(2):
                        c = half * 256
                        for hi in range(2):
                            r_sb = rpool.tile([1, 256], F32, tag="r")
                            nc.vector.reciprocal_approx_fast(
                                out=r_sb[:], in_=acc_sbs[hi][0:1, c : c + 256]
                            )
                            rb_ps = pp.tile(
                                [64, 256], F32, tag="pp", name=f"rbps{half}_{hi}"
                            )
                            nc.tensor.matmul(
                                rb_ps[:], lhsT=ones_sb[:], rhs=r_sb[:],
                                start=True, stop=True,
                            )
                            nc.vector.tensor_mul(
                                outT_sb[hi * 64 : (hi + 1) * 64, hp, c : c + 256],
                                acc_sbs[hi][64 : 64 + DH, c : c + 256],
                                rb_ps[:],
                            )
                        for mq in (half * 2, half * 2 + 1):
                            emit_wo_group(qp, outT_sb, mq)
                else:
                    for hi in range(2):
                        acc_sb = rpool.tile([128, 512], F32, tag="accsb")
                        nc.vector.tensor_copy(acc_sb[:], acc[hi][:])
                        r_sb = rpool.tile([1, 512], F32, tag="r")
                        nc.vector.reciprocal_approx_fast(
                            out=r_sb[:], in_=acc_sb[0:1, :]
                        )
                        row = qp * 4 + hp * 2 + hi
                        nc.sync.dma_start(r_dram[row : row + 1, :], r_sb[:])
                        rb_sb = rpool.tile([128, 512], F32, tag="rb")
                        src = r_dram[row : row + 1, :]
                        nc.sync.dma_start(
                            rb_sb[64:128, :],
                            bass.AP(
                                tensor=src.tensor,
                                offset=src.offset,
                                ap=[[0, 64]] + src.ap[1:],
                            ),
                        )
                        nc.vector.tensor_mul(
                            outT_sb[hi * 64 : (hi + 1) * 64, hp, :],
                            acc_sb[64 : 64 + DH, :],
                            rb_sb[64:128, :],
                        )
                if hp == 1 and qp < 3:
                    for mq in range(4):
                        pending_wo.append((qp, outT_sb, mq))

        assert not sched, f"unconsumed scheduled units: {sorted(sched)}"
        while pending_wo:
            emit_wo_group(*pending_wo.pop(0))

    nc.compile()
    return nc


def _get_program():
    global _PROGRAM
    if _PROGRAM is None:
        _PROGRAM = build_program()
    return _PROGRAM


def make_in_maps(Q, K, V, mask, W_q, W_k, W_v, W_o):
    bf = ml_dtypes.bfloat16
    Q, K, V = (np.asarray(a, np.float32) for a in (Q, K, V))
    W_q, W_k, W_v, W_o = (np.asarray(a, np.float32) for a in (W_q, W_k, W_v, W_o))
    mask = np.asarray(mask)
    in_maps = []
    for core in range(NCORES):
        b, hg = core // 4, core % 4
        c0 = hg * DL
        in_maps.append(
            {
                "xqT": np.ascontiguousarray(Q[b].T).astype(bf),
                "xkT": np.ascontiguousarray(K[b].T).astype(bf),
                "xvT": np.ascontiguousarray(V[b].T).astype(bf),
                "wq": np.ascontiguousarray(W_q[c0 : c0 + DL, :].T).astype(bf),
                "wk": np.ascontiguousarray(W_k[c0 : c0 + DL, :].T).astype(bf),
                "wv": np.ascontiguousarray(W_v[c0 : c0 + DL, :].T).astype(bf),
                "wo": np.ascontiguousarray(W_o[:, c0 : c0 + DL].T).astype(bf),
                "maskf": np.ascontiguousarray(
                    mask[b].reshape(SEQ // 128, 128).T
                ).astype(np.float32),
            }
        )
    return in_maps


def gather(results):
    out = np.zeros((B, SEQ, D), np.float32)
    for core in range(NCORES):
        out[core // 4] += results[core]["out_part"]
    return out


def kernel(Q, K, V, mask, W_q, W_k, W_v, W_o):
    from concourse.bass_utils import run_bass_kernel_spmd

    nc = _get_program()
    in_maps = make_in_maps(Q, K, V, mask, W_q, W_k, W_v, W_o)
    res = run_bass_kernel_spmd(nc, in_maps, list(range(NCORES))).results
    return gather(res)


# revision 18
# speedup vs baseline: 1.0605x; 1.0605x over previous
"""MultiHeadCrossAttention kernel for 8 trn2 NeuronCores.

Reference computation (fp32, per batch b):
    q = Q[b] @ W_q.T ; k = K[b] @ W_k.T ; v = V[b] @ W_v.T      (heads on columns)
    per head h: S = (q_h @ k_h.T) / 8 ; E = exp(S); A = E / E.sum(-1)
    out[b] = concat_h(A @ v_h) @ W_o.T ; rows with mask==0 zeroed

Sharding: 8 cores = (batch b in {0,1}) x (head-group hg in {0..3}, 4 heads each).
Each core computes a partial output  out_part[b] = concat(heads hg) @ W_o[:, cols].T
and the host sums the 4 partials per batch.

Schedule (v4): jointly limited by the PE (~137us of matmul stream) and the
Scalar engine (~137us of exp).  Dense same-shape matmul phases run at
~1.0x N-cycles while finely interleaved ones run ~20-100% slower, so the
big projection phases stay dense; only the lag-tolerant pieces (v-proj
chunks, later q-blocks) are emitted as units inside the attention loop:
  - DMAs split across both HWDGE rings in consumption order (K path first).
  - Pre-attention: full k-projection (dense), first q-block per head pair,
    v chunks 0-1.  exp starts ~30us in (vs ~60 for proj-everything-first).
  - v chunks 2-15 one per iteration (PV tolerates lag via the e-tile pool);
    remaining six q-blocks at iterations chosen well before their consumers.
  - W_o groups drain one per 4 iterations to avoid PE bursts at block
    boundaries.
  - Projection PSUM->SBUF copies run on Vector, never Scalar (exp only).

Per-core kernel layout (all matmul operands bf16, fp32 PSUM accumulation):
  - xqT/xkT/xvT: host-transposed [1024(in), 2048(seq)] so the contraction dim
    (model dim) lands on SBUF partitions.
  - qT/kT stored as [128, 2, 2048]: partition = local-head-dim % 128. Head pair
    hp occupies chunk hp; even head rows 0:64, odd head rows 64:128 - so the two
    scores matmuls of a pair use disjoint PE row groups and run concurrently.
  - Scores are computed transposed, ST[kpos, q], per 128-kpos chunk j; exp runs
    on ScalarE over a 2-bank [128, 1024] PSUM region (even|odd head halves) with
    the 1/8 scale fused.
  - PV: acc[128, 512] += [ones | 0*63 | v_h].T @ E  accumulated over j. Row 0
    is the softmax denominator; rows 64:128 the unnormalized output ([d, q]).
  - Normalize: copy acc to SBUF (frees the PSUM bank fast), approx reciprocal
    of row 0, broadcast across partitions via a DRAM bounce + step-0 read (or a
    PE outer-product for the final pair, keeping the last W_o off the DMA
    latency), one tensor_mul into the W_o lhsT layout (bf16).
  - W_o: final[q,:] accumulated over the 256 local dims; the mask (per-q 0/1)
    is applied by the PSUM->SBUF tensor_scalar multiply before the output DMA.
"""

import numpy as np
import ml_dtypes

import concourse.bass as bass
import concourse.bacc as bacc
import concourse.mybir as mybir
import concourse.tile as tile
from contextlib import ExitStack

F32 = mybir.dt.float32
BF16 = mybir.dt.bfloat16
AF = mybir.ActivationFunctionType

B = 2
SEQ = 2048          # Sq == Sk
D = 1024            # model dim
DL = 256            # local head dims per core (4 heads x 64)
HL = 4              # local heads
DH = 64             # head dim
NCORES = 8

_PROGRAM = None


def build_program():
    nc = bacc.Bacc("TRN2", target_bir_lowering=False)

    xqT = nc.declare_dram_parameter("xqT", [D, SEQ], BF16, isOutput=False)
    xkT = nc.declare_dram_parameter("xkT", [D, SEQ], BF16, isOutput=False)
    xvT = nc.declare_dram_parameter("xvT", [D, SEQ], BF16, isOutput=False)
    wq = nc.declare_dram_parameter("wq", [D, DL], BF16, isOutput=False)
    wk = nc.declare_dram_parameter("wk", [D, DL], BF16, isOutput=False)
    wv = nc.declare_dram_parameter("wv", [D, DL], BF16, isOutput=False)
    wo = nc.declare_dram_parameter("wo", [DL, D], BF16, isOutput=False)
    maskf = nc.declare_dram_parameter("maskf", [128, SEQ // 128], F32, isOutput=False)
    out_part = nc.declare_dram_parameter("out_part", [SEQ, D], F32, isOutput=True)

    r_dram = nc.dram_tensor("r_bounce", [16, 512], F32)  # recip bounce rows

    with tile.TileContext(nc) as tc, ExitStack() as ctx:
        const = ctx.enter_context(tc.tile_pool(name="const", bufs=1))
        proj = ctx.enter_context(tc.tile_pool(name="proj", bufs=1))
        xpool = ctx.enter_context(tc.tile_pool(name="xpool", bufs=10))
        epool = ctx.enter_context(tc.tile_pool(name="epool", bufs=8))
        opool = ctx.enter_context(tc.tile_pool(name="opool", bufs=4))
        ospool = ctx.enter_context(tc.tile_pool(name="ospool", bufs=4))
        rpool = ctx.enter_context(tc.tile_pool(name="rpool", bufs=4))
        pp = ctx.enter_context(tc.tile_pool(name="pp", bufs=2, space="PSUM"))
        stp = ctx.enter_context(tc.tile_pool(name="stp", bufs=2, space="PSUM"))
        accp = ctx.enter_context(tc.tile_pool(name="accp", bufs=2, space="PSUM"))

        # ---------------- constants + inputs, consumption-ordered ----------
        wk_sb = const.tile([128, 8, DL], BF16)
        wq_sb = const.tile([128, 8, DL], BF16)
        wv_sb = const.tile([128, 8, DL], BF16)
        wo_sb = const.tile([128, 2, D], BF16)
        mask_sb = const.tile([128, SEQ // 128], F32)
        ones_sb = const.tile([1, 64], F32)

        def dma_x(xT, half, quarter, name, eng):
            x_t = xpool.tile([128, 4, 1024], BF16, tag="x", name=name)
            eng.dma_start(
                x_t[:],
                xT[
                    quarter * 512 : (quarter + 1) * 512,
                    half * 1024 : (half + 1) * 1024,
                ].rearrange("(a p) q -> p a q", p=128),
            )
            return x_t

        # quarter 0 on the sync ring, quarter 1 on the scalar ring.
        nc.scalar.dma_start(wk_sb[:], wk[:].rearrange("(a p) d -> p a d", p=128))
        xk = [
            [dma_x(xkT, 0, 0, "xk00", nc.sync), dma_x(xkT, 0, 1, "xk01", nc.scalar)],
            [dma_x(xkT, 1, 0, "xk10", nc.sync), dma_x(xkT, 1, 1, "xk11", nc.scalar)],
        ]
        nc.scalar.dma_start(wq_sb[:], wq[:].rearrange("(a p) d -> p a d", p=128))
        xq0 = [dma_x(xqT, 0, 0, "xq00", nc.sync), dma_x(xqT, 0, 1, "xq01", nc.scalar)]
        nc.scalar.dma_start(wv_sb[:], wv[:].rearrange("(a p) d -> p a d", p=128))
        xv = [
            [dma_x(xvT, 0, 0, "xv00", nc.sync), dma_x(xvT, 0, 1, "xv01", nc.scalar)],
            [dma_x(xvT, 1, 0, "xv10", nc.sync), dma_x(xvT, 1, 1, "xv11", nc.scalar)],
        ]
        nc.scalar.dma_start(wo_sb[:], wo[:].rearrange("(a p) d -> p a d", p=128))
        nc.scalar.dma_start(mask_sb[:], maskf[:])
        xq1 = [dma_x(xqT, 1, 0, "xq10", nc.sync), dma_x(xqT, 1, 1, "xq11", nc.scalar)]
        nc.vector.memset(ones_sb[:], 1.0)

        kT0_sb = proj.tile([128, SEQ], BF16)
        kT1_sb = proj.tile([128, SEQ], BF16)
        kTs = (kT0_sb, kT1_sb)
        qTs = [
            [proj.tile([128, 1024], BF16, name=f"qT{dm}_{h}") for h in range(2)]
            for dm in range(2)
        ]
        xqs = (xq0, xq1)
        vaugs = [
            proj.tile([128, HL, 128], BF16, name=f"vaug{j}") for j in range(16)
        ]
        for j in range(16):
            nc.vector.memset(vaugs[j][:], 0.0)
            nc.vector.memset(vaugs[j][:, :, 0:1], 1.0)

        # ---------------- projection emitters ----------------
        # dense 512-col k/q block: dst[:, dst_col:dst_col+512] over 8 ki.
        def emit_kq_block(w_sb, x_pair, qc, dm, dst, dst_col, nm):
            ps = pp.tile([128, 512], F32, tag="pp", name=f"kq{nm}")
            for ki in range(8):
                nc.tensor.matmul(
                    ps[:],
                    lhsT=w_sb[:, ki, dm * 128 : (dm + 1) * 128],
                    rhs=x_pair[ki // 4][:, ki % 4, qc * 512 : (qc + 1) * 512],
                    start=(ki == 0),
                    stop=(ki == 7),
                )
            nc.vector.tensor_copy(dst[:, dst_col : dst_col + 512], ps[:])

        # v chunk unit: vaug[base+km] for all 4 local heads.
        def emit_v_unit(half, km):
            x_pair = xv[half]
            ps = pp.tile([128, 512], F32, tag="pp", name=f"v{half}_{km}")
            for ki in range(8):
                nc.tensor.matmul(
                    ps[:, 0:DL],
                    lhsT=x_pair[ki // 4][:, ki % 4, km * 128 : (km + 1) * 128],
                    rhs=wv_sb[:, ki, :],
                    start=(ki == 0),
                    stop=(ki == 7),
                )
            nc.vector.tensor_copy(
                vaugs[half * 8 + km][:, :, 64 : 64 + DH],
                ps[:, 0:DL].rearrange("p (h d) -> p h d", h=HL),
            )

        # ---------------- pre-attention (dense) ----------------
        # k projection for head pair 0 only (kT1 is not consumed until g64)
        for half in range(2):
            for qc in range(2):
                emit_kq_block(
                    wk_sb, xk[half], qc, 0, kTs[0],
                    half * 1024 + qc * 512, f"k{half}0{qc}",
                )
        # first q block for head pair 0 (q 0:512)
        emit_kq_block(wq_sb, xq0, 0, 0, qTs[0][0], 0, "q000")
        # v chunks 0, 1
        emit_v_unit(0, 0)
        emit_v_unit(0, 1)

        # ---------------- scheduled units ----------------
        # g = hp*64 + qp*16 + j; unit emitted at END of iteration g, always
        # before its first consumer's emission.
        sched = {}

        def at(g, fn):
            sched.setdefault(g, []).append(fn)

        for km in range(2, 8):      # vaug 2..7, consumed by PV hp0 at g=km
            at(km - 2, lambda km=km: emit_v_unit(0, km))
        for km in range(8):         # vaug 8..15, consumed at g=8+km
            at(6 + km, lambda km=km: emit_v_unit(1, km))
        # qT0 rest: consumed at qp1/qp2/qp3 of phase 1 (g16/g32/g48)
        at(4, lambda: emit_kq_block(wq_sb, xq0, 1, 0, qTs[0][0], 512, "q010"))
        at(20, lambda: emit_kq_block(wq_sb, xq1, 0, 0, qTs[0][1], 0, "q100"))
        at(36, lambda: emit_kq_block(wq_sb, xq1, 1, 0, qTs[0][1], 512, "q110"))
        # kT1: consumed from g64 (phase 2), kpos quarter at g64/g68/g72/g76
        at(24, lambda: emit_kq_block(wk_sb, xk[0], 0, 1, kTs[1], 0, "k100"))
        at(28, lambda: emit_kq_block(wk_sb, xk[0], 1, 1, kTs[1], 512, "k101"))
        at(40, lambda: emit_kq_block(wk_sb, xk[1], 0, 1, kTs[1], 1024, "k110"))
        at(44, lambda: emit_kq_block(wk_sb, xk[1], 1, 1, kTs[1], 1536, "k111"))
        # qT1: consumed at g64/g80/g96/g112
        at(48, lambda: emit_kq_block(wq_sb, xq0, 0, 1, qTs[1][0], 0, "q001"))
        at(52, lambda: emit_kq_block(wq_sb, xq0, 1, 1, qTs[1][0], 512, "q011"))
        at(56, lambda: emit_kq_block(wq_sb, xq1, 0, 1, qTs[1][1], 0, "q101"))
        at(60, lambda: emit_kq_block(wq_sb, xq1, 1, 1, qTs[1][1], 512, "q111"))

        # ---------------- attention + output projection ----------------
        def emit_wo_group(qp, outT_sb, mq):
            qg = qp * 4 + mq
            o_sb = ospool.tile([128, 1024], F32, tag="o", name=f"wo_o{qp}_{mq}")
            ps = [
                pp.tile([128, 512], F32, tag="pp", name=f"wops{qp}_{mq}_{oc}")
                for oc in range(2)
            ]
            for kc in range(2):
                for oc in range(2):
                    nc.tensor.matmul(
                        ps[oc][:],
                        lhsT=outT_sb[:, kc, mq * 128 : (mq + 1) * 128],
                        rhs=wo_sb[:, kc, oc * 512 : (oc + 1) * 512],
                        start=(kc == 0),
                        stop=(kc == 1),
                    )
            for oc in range(2):
                nc.vector.tensor_scalar_mul(
                    o_sb[:, oc * 512 : (oc + 1) * 512],
                    ps[oc][:],
                    mask_sb[:, qg : qg + 1],
                )
            nc.sync.dma_start(out_part[qg * 128 : (qg + 1) * 128, :], o_sb[:])

        pending_wo = []
        outTs = {}

        for hp in range(2):  # head pairs (2*hp, 2*hp+1) -- OUTER
            for qp in range(4):  # 512-query blocks
                q0 = qp * 512
                if hp == 0:
                    outTs[qp] = opool.tile(
                        [128, 2, 512], BF16, tag="outT", name=f"outT{qp}"
                    )
                outT_sb = outTs[qp]
                acc = [
                    accp.tile([128, 512], F32, tag="acc", name=f"acc{qp}_{hp}_{i}")
                    for i in range(2)
                ]
                for j in range(16):  # 128-key chunks
                    g = hp * 64 + qp * 16 + j
                    st = stp.tile([128, 1024], F32, tag="st")
                    for hi in range(2):  # even/odd head -> PE rows 0:64 / 64:128
                        r0 = hi * 64
                        nc.tensor.matmul(
                            st[:, hi * 512 : (hi + 1) * 512],
                            lhsT=kTs[hp][r0 : r0 + 64, j * 128 : (j + 1) * 128],
                            rhs=qTs[hp][q0 // 1024][
                                r0 : r0 + 64, (q0 % 1024) : (q0 % 1024) + 512
                            ],
                            start=True,
                            stop=True,
                        )
                    e_t = epool.tile([128, 1024], BF16, tag="e")
                    nc.scalar.activation(out=e_t[:], in_=st[:], func=AF.Exp, scale=0.125)
                    for hi in range(2):
                        h = 2 * hp + hi
                        nc.tensor.matmul(
                            acc[hi][:],
                            lhsT=vaugs[j][:, h, :],
                            rhs=e_t[:, hi * 512 : (hi + 1) * 512],
                            start=(j == 0),
                            stop=(j == 15),
                        )
                    for fn in sched.pop(g, ()):
                        fn()
                    if j % 4 == 1 and pending_wo:
                        emit_wo_group(*pending_wo.pop(0))
                if qp == 3 and hp == 1:
                    # tail: split the last normalize + W_o into two 256-q
                    # halves so W_o starts as soon as its half is ready; the
                    # reciprocal broadcast runs on-chip (PE outer product) to
                    # skip the DRAM round trip.
                    acc_sbs = []
                    for hi in range(2):
                        acc_sb = rpool.tile([128, 512], F32, tag="accsb")
                        nc.vector.tensor_copy(acc_sb[:], acc[hi][:])
                        acc_sbs.append(acc_sb)
                    for half in range(2):
                        c = half * 256
                        for hi in range(2):
                            r_sb = rpool.tile([1, 256], F32, tag="r")
                            nc.vector.reciprocal_approx_fast(
                                out=r_sb[:], in_=acc_sbs[hi][0:1, c : c + 256]
                            )
                            rb_ps = pp.tile(
                                [64, 256], F32, tag="pp", name=f"rbps{half}_{hi}"
                            )
                            nc.tensor.matmul(
                                rb_ps[:], lhsT=ones_sb[:], rhs=r_sb[:],
                                start=True, stop=True,
                            )
                            nc.vector.tensor_mul(
                                outT_sb[hi * 64 : (hi + 1) * 64, hp, c : c + 256],
                                acc_sbs[hi][64 : 64 + DH, c : c + 256],
                                rb_ps[:],
                            )
                        for mq in (half * 2, half * 2 + 1):
                            emit_wo_group(qp, outT_sb, mq)
                else:
                    for hi in range(2):
                        acc_sb = rpool.tile([128, 512], F32, tag="accsb")
                        nc.vector.tensor_copy(acc_sb[:], acc[hi][:])
                        r_sb = rpool.tile([1, 512], F32, tag="r")
                        nc.vector.reciprocal_approx_fast(
                            out=r_sb[:], in_=acc_sb[0:1, :]
                        )
                        row = qp * 4 + hp * 2 + hi
                        nc.sync.dma_start(r_dram[row : row + 1, :], r_sb[:])
                        rb_sb = rpool.tile([128, 512], F32, tag="rb")
                        src = r_dram[row : row + 1, :]
                        nc.sync.dma_start(
                            rb_sb[64:128, :],
                            bass.AP(
                                tensor=src.tensor,
                                offset=src.offset,
                                ap=[[0, 64]] + src.ap[1:],
                            ),
                        )
                        nc.vector.tensor_mul(
                            outT_sb[hi * 64 : (hi + 1) * 64, hp, :],
                            acc_sb[64 : 64 + DH, :],
                            rb_sb[64:128, :],
                        )
                if hp == 1 and qp < 3:
                    for mq in range(4):
                        pending_wo.append((qp, outT_sb, mq))

        assert not sched, f"unconsumed scheduled units: {sorted(sched)}"
        while pending_wo:
            emit_wo_group(*pending_wo.pop(0))

    nc.compile()
    return nc


def _get_program():
    global _PROGRAM
    if _PROGRAM is None:
        _PROGRAM = build_program()
    return _PROGRAM


def make_in_maps(Q, K, V, mask, W_q, W_k, W_v, W_o):
    bf = ml_dtypes.bfloat16
    Q, K, V = (np.asarray(a, np.float32) for a in (Q, K, V))
    W_q, W_k, W_v, W_o = (np.asarray(a, np.float32) for a in (W_q, W_k, W_v, W_o))
    mask = np.asarray(mask)
    in_maps = []
    for core in range(NCORES):
        b, hg = core // 4, core % 4
        c0 = hg * DL
        in_maps.append(
            {
                "xqT": np.ascontiguousarray(Q[b].T).astype(bf),
                "xkT": np.ascontiguousarray(K[b].T).astype(bf),
                "xvT": np.ascontiguousarray(V[b].T).astype(bf),
                "wq": np.ascontiguousarray(W_q[c0 : c0 + DL, :].T).astype(bf),
                "wk": np.ascontiguousarray(W_k[c0 : c0 + DL, :].T).astype(bf),
                "wv": np.ascontiguousarray(W_v[c0 : c0 + DL, :].T).astype(bf),
                "wo": np.ascontiguousarray(W_o[:, c0 : c0 + DL].T).astype(bf),
                "maskf": np.ascontiguousarray(
                    mask[b].reshape(SEQ // 128, 128).T
                ).astype(np.float32),
            }
        )
    return in_maps


def gather(results):
    out = np.zeros((B, SEQ, D), np.float32)
    for core in range(NCORES):
        out[core // 4] += results[core]["out_part"]
    return out


def kernel(Q, K, V, mask, W_q, W_k, W_v, W_o):
    from concourse.bass_utils import run_bass_kernel_spmd

    nc = _get_program()
    in_maps = make_in_maps(Q, K, V, mask, W_q, W_k, W_v, W_o)
    res = run_bass_kernel_spmd(nc, in_maps, list(range(NCORES))).results
    return gather(res)
